# revision 5
# baseline (speedup 1.0000x reference)
"""Trainium2 Bass kernel for nn_Attention (B=1, C=64, 12x12x12 spatial, 32 heads, head_dim=2).

Sharding: 32 heads split across 8 cores (4 heads/core). Each core computes
qkv projection for its heads, head-local attention (flash-style: S^T chunks
-> exp on ScalarE -> U/Z accumulation via matmul with V'=[V,1]), divides,
then applies its slice of w_proj rows to produce a partial output summed on
the host (tensor-parallel unshard) with bias/8 folded per core.

The kernel is ScalarE(exp)-bound: 4 heads x 1728^2 scores = 11.9M exps
per core at 1 elem/cycle/lane. Everything else is structured to keep ACT
at ~100% duty:
 - PE emission order per key-chunk kc is  S(kc) -> U(kc-1)  (software
   pipeline one chunk ahead), so scores for the next exp are always ready.
 - V' and qkv for later spans are emitted at loop boundaries where PE/DVE
   have slack under the ACT-bound steady state.
 - Input DMAs go out on four different engine queues in parallel; a tiny
   warmup exp triggers the ~2.7us ACT table load during the DMAs.
 - Tail is minimized: 128-token proj chunks for the first 1024 tokens are
   emitted inside the qt1 loop; the final 704-token divide runs in two
   halves so proj/DMA overlap the second reciprocal.

Uses bacc.Bacc (not plain Bass): its compile() runs
move_matmul_waits_to_ldweights + generate_event_semaphores, which the
TRN2 one-wait-per-instruction ISA constraint requires for Tile kernels.

Self-contained: hardcodes all shapes.
"""

import numpy as np
import ml_dtypes

import concourse.bass as bass
import concourse.bacc as bacc
import concourse.mybir as mybir
from concourse import tile
from concourse.bass_utils import run_bass_kernel_spmd

C = 64
N = 1728  # 12*12*12
NCORES = 8
HLOC = 4          # heads per core
SCALE = float(2.0 ** -0.5)

# key chunks: 13x128 + 64
KCS = [(i * 128, 128) for i in range(13)] + [(1664, 64)]
NKC = len(KCS)
# query tiles: big first tile, smaller second so the un-overlappable tail
# (reciprocal is FD-bound at 8 cyc/elem) is short
QTS = [(0, 1024), (1024, 704)]
# proj token chunks of 128 (last 64); chunks 0..7 lie inside query tile 0
TCS = [(i * 128, 128) for i in range(13)] + [(1664, 64)]
TC_SPLIT = 8

F32 = mybir.dt.float32
BF16 = mybir.dt.bfloat16


def _sub_mms(qn):
    out = []
    o = 0
    while o < qn:
        n = min(512, qn - o)
        out.append((o, n))
        o += n
    return out


def build_nc():
    nc = bacc.Bacc(None)

    x2 = nc.declare_dram_parameter("x2", [C, N], BF16, isOutput=False)
    wqkv = nc.declare_dram_parameter("wqkv", [C, 6 * HLOC], BF16, isOutput=False)
    wp = nc.declare_dram_parameter("wp", [2 * HLOC + 1, C], F32, isOutput=False)
    y = nc.declare_dram_parameter("y", [N, C], F32, isOutput=True)

    with tile.TileContext(nc) as tc:
        with (
            tc.tile_pool(name="const", bufs=1) as cpool,
            tc.tile_pool(name="epool", bufs=9) as epool,
            tc.tile_pool(name="upool", bufs=2) as upool,
            tc.tile_pool(name="ps_s", bufs=3, space=bass.MemorySpace.PSUM) as ps_s,
            tc.tile_pool(name="ps_u", bufs=1, space=bass.MemorySpace.PSUM) as ps_u,
        ):
            x_sb = cpool.tile([C, N], BF16, name="x_sb")
            w_sb = cpool.tile([C, 6 * HLOC], BF16, name="w_sb")
            wp_sb = cpool.tile([2 * HLOC + 1, C], F32, name="wp_sb")
            qT = cpool.tile([128, N], BF16, name="qT")
            kT = cpool.tile([128, N], BF16, name="kT")
            vp = cpool.tile([128, NKC * 3 * HLOC], BF16, name="vp")
            ot = cpool.tile([2 * HLOC + 1, N], F32, name="ot")
            ybig = cpool.tile([128, len(TCS) * C], F32, name="ybig")
            ybv = ybig[:].rearrange("p (t c) -> p t c", c=C)
            wrm = cpool.tile([1, 8], BF16, name="wrm")

            # ACT table warmup: the ~2.7us exp table load runs during DMAs
            nc.gpsimd.memset(wrm[:], 0.0)
            nc.scalar.activation(
                wrm[:], wrm[:], mybir.ActivationFunctionType.Exp
            )

            # input DMAs on four queues in parallel, x halves first
            nc.sync.dma_start(out=x_sb[:, 0:1024], in_=x2[:, 0:1024])
            nc.scalar.dma_start(out=x_sb[:, 1024:N], in_=x2[:, 1024:N])
            nc.gpsimd.dma_start(out=w_sb[:], in_=wqkv[:])
            nc.gpsimd.dma_start(out=wp_sb[:], in_=wp[:])

            # ones row for proj bias (rows 0..7 overwritten by attention out)
            nc.gpsimd.memset(ot[:, :], 1.0)
            vp_v = vp[:].rearrange("p (a b c) -> p a b c", b=HLOC, c=3)
            nc.gpsimd.memset(vp_v[:, :, :, 2:3], 1.0)

            wq_sl = w_sb[:, 0 : 2 * HLOC]
            wk_sl = w_sb[:, 2 * HLOC : 4 * HLOC]
            wv_sl = w_sb[:, 4 * HLOC : 6 * HLOC]

            # ---- V' in groups of 4 key chunks (spread over boundaries) ----
            def emit_vprime_group(g):
                kcs = [kc for kc in range(4 * g, min(4 * g + 4, NKC))]
                psv = ps_s.tile([128, 1024], F32, tag="s", name="ps_v")
                for kc in kcs:
                    ko, kn = KCS[kc]
                    nc.tensor.matmul(
                        psv[:kn, 8 * (kc - 4 * g) : 8 * (kc - 4 * g) + 2 * HLOC],
                        x_sb[:, ko : ko + kn],
                        wv_sl,
                        start=True, stop=True,
                    )
                vsrc = psv[:, 0 : 8 * len(kcs)].rearrange(
                    "p (kc h d) -> p kc h d", h=HLOC, d=2
                )
                nc.vector.tensor_copy(
                    vp_v[:, 4 * g : 4 * g + len(kcs), :, 0:2], vsrc
                )

            def qkv_tile(w_sl, dst, off, qn, heads=range(HLOC)):
                """Per-head matmuls (rows at partitions 32h) + per-head copy."""
                ps = ps_s.tile([128, 1024], F32, tag="s", name="ps_qkv")
                for h in heads:
                    for (o, n_) in _sub_mms(qn):
                        nc.tensor.matmul(
                            ps[32 * h : 32 * h + 2, o : o + n_],
                            w_sl[:, 2 * h : 2 * h + 2],
                            x_sb[:, off + o : off + o + n_],
                            start=True, stop=True,
                            tile_position=(0, 32 * h),
                        )
                    nc.vector.tensor_copy(
                        dst[32 * h : 32 * h + 2, off : off + qn],
                        ps[32 * h : 32 * h + 2, :qn],
                    )

            # q half 0 / first k cols emitted per-head just before each
            # head's first S matmul so exp(0,h0) starts ASAP
            def pre_s0(kc, h):
                if kc == 0:
                    qkv_tile(wq_sl, qT, 0, 1024, heads=[h])
                    qkv_tile(wk_sl, kT, 0, 512, heads=[h])

            def emit_U(pu, kc, es, qo, qn):
                ko, kn = KCS[kc]
                for h in range(HLOC):
                    for (o, n_) in _sub_mms(qn):
                        nc.tensor.matmul(
                            pu[32 * h : 32 * h + 3, o : o + n_],
                            vp_v[:kn, kc, h, :],
                            es[h][:kn, o : o + n_],
                            start=(kc == 0), stop=(kc == NKC - 1),
                            tile_position=(0, 32 * h),
                        )

            def main_loop(qo, qn, boundary_work, pre_s=None):
                pu = ps_u.tile([128, 1024], F32, tag="pu", name="pu")
                es_prev = None
                for kc, (ko, kn) in enumerate(KCS):
                    es = []
                    for h in range(HLOC):
                        if pre_s is not None:
                            pre_s(kc, h)
                        ps = ps_s.tile([128, 1024], F32, tag="s", name="ps_att")
                        for (o, n_) in _sub_mms(qn):
                            nc.tensor.matmul(
                                ps[:kn, o : o + n_],
                                kT[32 * h : 32 * h + 2, ko : ko + kn],
                                qT[32 * h : 32 * h + 2, qo + o : qo + o + n_],
                                start=True, stop=True,
                                tile_position=(32 * h, 0),
                            )
                        e = epool.tile([128, 1024], BF16, tag="e", name="e")
                        nc.scalar.activation(
                            e[:kn, :qn], ps[:kn, :qn],
                            mybir.ActivationFunctionType.Exp, scale=SCALE,
                        )
                        es.append(e)
                    # U one key-chunk behind: PE stream is S(kc), U(kc-1),
                    # S(kc+1), U(kc)... so scores for exp are always ready
                    if kc > 0:
                        emit_U(pu, kc - 1, es_prev, qo, qn)
                    work = boundary_work.get(kc)
                    if work:
                        work()
                    es_prev = es
                emit_U(pu, NKC - 1, es_prev, qo, qn)
                return pu

            def divide_and_store(pu, qo, qn, qoff=0, last=False):
                """O^T rows 2h+d of `ot` <- U rows / Z row (per head).

                qoff/qn select a sub-range of pu's columns (origin qo)."""
                if last:
                    usrc = pu[:, qoff : qoff + qn]
                else:
                    u_sb = upool.tile([128, 1024], F32, tag="u_sb", name="u_sb")
                    nc.vector.tensor_copy(u_sb[:, :qn], pu[:, qoff : qoff + qn])
                    usrc = u_sb[:, :qn]
                zrec = upool.tile([128, 1024], F32, tag="zrec", name="zrec")
                nc.vector.reciprocal(zrec[:, :qn], usrc)
                zz = upool.tile([128, 1024], F32, tag="zz", name="zz")
                zzv_ = zz[:, :qn].rearrange("(h g) f -> h g f", g=32)
                zrv_ = zrec[:, :qn].rearrange("(h g) f -> h g f", g=32)
                nc.sync.dma_start(out=zzv_[:, 0, :], in_=zrv_[:, 2, :])
                nc.gpsimd.dma_start(out=zzv_[:, 1, :], in_=zrv_[:, 2, :])
                osp = upool.tile([128, 1024], F32, tag="osp", name="osp")
                nc.vector.tensor_mul(osp[:, :qn], usrc, zz[:, :qn])
                ospv = osp[:, :qn].rearrange("(h g) f -> h g f", g=32)
                otv = ot[0 : 2 * HLOC, qo + qoff : qo + qoff + qn].rearrange(
                    "(h g) f -> h g f", g=2
                )
                nc.sync.dma_start(out=otv[:, 0, :], in_=ospv[:, 0, :])
                nc.gpsimd.dma_start(out=otv[:, 1, :], in_=ospv[:, 1, :])

            def proj_chunks(ts_):
                for t in ts_:
                    to, tn = TCS[t]
                    py = ps_s.tile([128, 1024], F32, tag="s", name="py")
                    nc.tensor.matmul(
                        py[:tn, 0:C], ot[:, to : to + tn], wp_sb[:],
                        start=True, stop=True,
                    )
                    nc.vector.tensor_copy(ybv[:tn, t, :], py[:tn, 0:C])

            # qt0 loop: V' groups + remaining qkv at early boundaries
            # (PE has slack under the ACT-bound steady state)
            bw0 = {
                0: lambda: emit_vprime_group(0),
                1: lambda: (emit_vprime_group(1),
                            qkv_tile(wk_sl, kT, 512, 512, heads=[0, 1])),
                2: lambda: (emit_vprime_group(2),
                            qkv_tile(wk_sl, kT, 512, 512, heads=[2, 3])),
                3: lambda: (emit_vprime_group(3),
                            qkv_tile(wq_sl, qT, 1024, 704, heads=[0, 1])),
                4: lambda: qkv_tile(wq_sl, qT, 1024, 704, heads=[2, 3]),
                5: lambda: qkv_tile(wk_sl, kT, 1024, 704, heads=[0, 1]),
                6: lambda: qkv_tile(wk_sl, kT, 1024, 704, heads=[2, 3]),
            }
            pu0 = main_loop(0, 1024, bw0, pre_s=pre_s0)
            divide_and_store(pu0, 0, 1024)

            # qt1 loop: qt0's proj + first y DMA emitted at boundaries
            # (after qt0's divide chain has drained on DVE/DMA)
            def y_dma0():
                yv0 = y[0:1024, :].rearrange("(t i) c -> i t c", i=128)
                nc.sync.dma_start(out=yv0, in_=ybv[:128, 0:TC_SPLIT, :])

            bw1 = {
                2: lambda: proj_chunks([0, 1]),
                3: lambda: proj_chunks([2, 3]),
                4: lambda: proj_chunks([4, 5]),
                5: lambda: proj_chunks([6, 7]),
                6: y_dma0,
            }
            pu1 = main_loop(1024, 704, bw1)

            # tail: divide the last 704 tokens in two halves so the second
            # reciprocal overlaps the first half's proj
            divide_and_store(pu1, 1024, 384, qoff=0, last=True)
            proj_chunks([8, 9, 10])
            divide_and_store(pu1, 1024, 320, qoff=384, last=True)
            yv1 = y[1024:1408, :].rearrange("(t i) c -> i t c", i=128)
            nc.sync.dma_start(out=yv1, in_=ybv[:128, 8:11, :])
            proj_chunks([11, 12, 13])
            yv2a = y[1408:1664, :].rearrange("(t i) c -> i t c", i=128)
            nc.sync.dma_start(out=yv2a, in_=ybv[:128, 11:13, :])
            nc.gpsimd.dma_start(out=y[1664:1728, :], in_=ybv[:64, 13, :])

    return nc


_NC = None


def _get_nc():
    global _NC
    if _NC is None:
        _NC = build_nc()
        _NC.finalize()
    return _NC


def make_in_maps(x, w_qkv, w_proj, b_proj):
    x2 = np.ascontiguousarray(x.reshape(C, N)).astype(ml_dtypes.bfloat16)
    in_maps = []
    for c in range(NCORES):
        sl = slice(8 * c, 8 * c + 8)
        wq = w_qkv[sl, :].T
        wk = w_qkv[64 + 8 * c : 64 + 8 * c + 8, :].T
        wv = w_qkv[128 + 8 * c : 128 + 8 * c + 8, :].T
        wall = np.ascontiguousarray(
            np.concatenate([wq, wk, wv], axis=1)
        ).astype(ml_dtypes.bfloat16)
        wpm = np.concatenate(
            [w_proj[:, sl].T, (b_proj / NCORES)[None, :]], axis=0
        ).astype(np.float32)
        in_maps.append(
            {"x2": x2, "wqkv": wall, "wp": np.ascontiguousarray(wpm)}
        )
    return in_maps


def run(x, w_qkv, w_proj, b_proj, trace=False, **kw):
    nc = _get_nc()
    in_maps = make_in_maps(x, w_qkv, w_proj, b_proj)
    res = run_bass_kernel_spmd(
        nc, in_maps, core_ids=list(range(NCORES)), trace=trace, **kw
    )
    y = np.zeros((N, C), np.float32)
    for r in res.results:
        y += r["y"]
    return y.reshape(1, 12, 12, 12, C), res


def kernel(x, w_qkv, w_proj, b_proj):
    out, _ = run(
        np.asarray(x), np.asarray(w_qkv), np.asarray(w_proj), np.asarray(b_proj)
    )
    return out


# revision 14
# speedup vs baseline: 1.6074x; 1.6074x over previous
"""Trainium2 Bass kernel for nn_Attention (B=1, C=64, 12x12x12 spatial, 32 heads, head_dim=2).

Sharding: 32 heads split across 8 cores (4 heads/core). Each core computes
qkv projection for its heads, head-local attention (flash-style: S^T chunks
-> exp on ScalarE -> U/Z accumulation via matmul with V'=[V,1]), divides,
then applies its slice of w_proj rows to produce a partial output summed on
the host (tensor-parallel unshard) with bias/8 folded per core.

The kernel is ScalarE(exp)-bound: 4 heads x 1728^2 scores = 11.9M exps per
core at 1 elem/cycle/lane (~78us of pure FD time).  Structure:

 - Work is a stream of (key-chunk, head) "slots", each a [kn<=128, qn=512]
   score tile (one PSUM bank).  Slots are grouped 3 per "unit" = one
   [128, 1536] PSUM tile; ONE ACTIVATE per unit (FD=1536) amortizes the
   ~260ns per-instruction overhead that dominated the per-(kc,h) version.
 - The 3 S matmuls of a unit hit different PE row-strips (tile_position
   32h) and write disjoint banks of one tile, so they run concurrently;
   same for the U matmuls (col-strips).  PE stays well under the ACT rate.
 - PSUM: 2 unit buffers (2x3 banks) + 2 U accumulators (2x1 bank) = 8.
   Every loop boundary allocates PSUM scratch tiles in PAIRS so
   consecutive units always land on different ring buffers.
 - Query dim is processed in tiles of 512 (x3) + 192; U/Z accumulate per
   qtile; divide + w_proj run at the next qtile's early boundaries.
 - Input DMAs go out on three queues in parallel; a tiny warmup exp
   triggers the ~2.7us ACT table load during the DMAs.

Uses bacc.Bacc (not plain Bass): its compile() runs
move_matmul_waits_to_ldweights + generate_event_semaphores, which the
TRN2 one-wait-per-instruction ISA constraint requires for Tile kernels.

Self-contained: hardcodes all shapes.
"""

import numpy as np
import ml_dtypes

import concourse.bass as bass
import concourse.bacc as bacc
import concourse.mybir as mybir
from concourse import tile
from concourse.bass_utils import run_bass_kernel_spmd

C = 64
N = 1728  # 12*12*12
NCORES = 8
HLOC = 4          # heads per core
SCALE = float(2.0 ** -0.5)

# key chunks: 13x128 + one padded 64+64 chunk (keys 1728:1792 are zero-pad:
# zero k columns -> score 0 -> E=1, and V' rows are zeroed -> contribute
# nothing to U or Z; keeps every S tile a full 128 rows so exp never reads
# uninitialized PSUM)
NK = 1792  # padded key count
KCS = [(i * 128, 128) for i in range(14)]
NKC = len(KCS)
QTS = [(0, 512), (512, 512), (1024, 512), (1536, 192)]
SLOTS = [(kc, h) for kc in range(NKC) for h in range(HLOC)]  # 56
UNITS = [SLOTS[i : i + 3] for i in range(0, len(SLOTS), 3)]  # 18x3 + 1x2
# proj token chunks of 128 (last 64)
TCS = [(i * 128, 128) for i in range(13)] + [(1664, 64)]

F32 = mybir.dt.float32
BF16 = mybir.dt.bfloat16
EXP = mybir.ActivationFunctionType.Exp


def build_nc():
    nc = bacc.Bacc(None)

    x2 = nc.declare_dram_parameter("x2", [C, N], BF16, isOutput=False)
    wqkv = nc.declare_dram_parameter("wqkv", [C, 6 * HLOC], BF16, isOutput=False)
    wp = nc.declare_dram_parameter("wp", [2 * HLOC + 1, C], F32, isOutput=False)
    y = nc.declare_dram_parameter("y", [N, C], F32, isOutput=True)

    with tile.TileContext(nc) as tc:
        with (
            tc.tile_pool(name="const", bufs=1) as cpool,
            tc.tile_pool(name="epool", bufs=4) as epool,
            tc.tile_pool(name="upool", bufs=2) as upool,
            tc.tile_pool(name="ps_s", bufs=2, space=bass.MemorySpace.PSUM) as ps_s,
            tc.tile_pool(name="ps_u", bufs=2, space=bass.MemorySpace.PSUM) as ps_u,
        ):
            x_sb = cpool.tile([C, N], BF16, name="x_sb")
            w_sb = cpool.tile([C, 6 * HLOC], BF16, name="w_sb")
            wp_sb = cpool.tile([2 * HLOC + 1, C], F32, name="wp_sb")
            qT = cpool.tile([128, N], BF16, name="qT")
            kT = cpool.tile([128, NK], BF16, name="kT")
            vp = cpool.tile([128, NKC * 3 * HLOC], BF16, name="vp")
            ot = cpool.tile([2 * HLOC + 1, N], F32, name="ot")
            ybig = cpool.tile([128, len(TCS) * C], F32, name="ybig")
            ybv = ybig[:].rearrange("p (t c) -> p t c", c=C)
            wrm = cpool.tile([1, 8], BF16, name="wrm")

            # ACT table warmup: the ~2.7us exp table load runs during DMAs
            nc.gpsimd.memset(wrm[:], 0.0)
            nc.scalar.activation(wrm[:], wrm[:], EXP)

            # input DMAs on three queues in parallel; first x chunk gates
            # the first S matmuls, so it goes out first on its own queue
            nc.sync.dma_start(out=x_sb[:, 0:512], in_=x2[:, 0:512])
            nc.sync.dma_start(out=x_sb[:, 512:1024], in_=x2[:, 512:1024])
            nc.scalar.dma_start(out=x_sb[:, 1024:N], in_=x2[:, 1024:N])
            nc.gpsimd.dma_start(out=w_sb[:], in_=wqkv[:])
            nc.gpsimd.dma_start(out=wp_sb[:], in_=wp[:])

            # ones row for proj bias (rows 0..7 overwritten by attention out)
            nc.gpsimd.memset(ot[:, :], 1.0)
            vp_v = vp[:].rearrange("p (a b c) -> p a b c", b=HLOC, c=3)
            nc.gpsimd.memset(vp_v[:, :, :, 2:3], 1.0)
            # zero-pad: k columns for pad keys and V' pad rows of last chunk
            nc.gpsimd.memset(kT[:, N:NK], 0.0)
            nc.gpsimd.memset(vp_v[64:128, NKC - 1 : NKC, :, :], 0.0)

            wq_sl = w_sb[:, 0 : 2 * HLOC]
            wk_sl = w_sb[:, 2 * HLOC : 4 * HLOC]
            wv_sl = w_sb[:, 4 * HLOC : 6 * HLOC]

            # ---- V' in groups of 4 key chunks; ONE ps_s ring slot each.
            # The last chunk only has 64 real keys (pad rows stay zero). ----
            def emit_vprime_group(g):
                kcs = list(range(4 * g, min(4 * g + 4, NKC)))
                psv = ps_s.tile([128, 1536], F32, tag="s", name="ps_v")
                rows = 128
                for i, kc in enumerate(kcs):
                    ko, kn = KCS[kc]
                    kr = min(kn, N - ko)  # real (non-pad) keys
                    rows = min(rows, kr)
                    nc.tensor.matmul(
                        psv[:kr, 8 * i : 8 * i + 2 * HLOC],
                        x_sb[:, ko : ko + kr],
                        wv_sl,
                        start=True, stop=True,
                    )
                vsrc = psv[:rows, 0 : 8 * len(kcs)].rearrange(
                    "p (kc h d) -> p kc h d", h=HLOC, d=2
                )
                nc.vector.tensor_copy(
                    vp_v[:rows, 4 * g : 4 * g + len(kcs), :, 0:2], vsrc
                )
                if rows < 128:
                    # full-row chunks of this group copied separately
                    vsrc2 = psv[rows:128, 0 : 8 * (len(kcs) - 1)].rearrange(
                        "p (kc h d) -> p kc h d", h=HLOC, d=2
                    )
                    nc.vector.tensor_copy(
                        vp_v[rows:128, 4 * g : 4 * g + len(kcs) - 1, :, 0:2],
                        vsrc2,
                    )

            def qkv_tile(w_sl, dst, off, qn, heads):
                """Per-head matmuls (rows at partitions 32h) + copies.
                One ps_s ring slot per call."""
                ps = ps_s.tile([128, 1536], F32, tag="s", name="ps_qkv")
                for h in heads:
                    o = 0
                    while o < qn:
                        n_ = min(512, qn - o)
                        nc.tensor.matmul(
                            ps[32 * h : 32 * h + 2, o : o + n_],
                            w_sl[:, 2 * h : 2 * h + 2],
                            x_sb[:, off + o : off + o + n_],
                            start=True, stop=True,
                            tile_position=(0, 32 * h),
                        )
                        o += n_
                    nc.vector.tensor_copy(
                        dst[32 * h : 32 * h + 2, off : off + qn],
                        ps[32 * h : 32 * h + 2, :qn],
                    )

            def emit_U(pu, e, unit, qn):
                for j, (kc, h) in enumerate(unit):
                    ko, kn = KCS[kc]
                    nc.tensor.matmul(
                        pu[32 * h : 32 * h + 3, 0:qn],
                        vp_v[:kn, kc, h, :],
                        e[0:kn, 512 * j : 512 * j + qn],
                        start=(kc == 0), stop=(kc == NKC - 1),
                        tile_position=(0, 32 * h),
                    )

            def divide_and_store(pu, qo, qn, last=False):
                """O^T rows 2h+d of `ot` <- U rows / Z row (per head)."""
                if last:
                    usrc = pu[:, :qn]
                else:
                    u_sb = upool.tile([128, 512], F32, tag="u_sb", name="u_sb")
                    nc.vector.tensor_copy(u_sb[:, :qn], pu[:, :qn])
                    usrc = u_sb[:, :qn]
                zrec = upool.tile([128, 512], F32, tag="zrec", name="zrec")
                nc.vector.reciprocal(zrec[:, :qn], usrc)
                zz = upool.tile([128, 512], F32, tag="zz", name="zz")
                zzv_ = zz[:, :qn].rearrange("(h g) f -> h g f", g=32)
                zrv_ = zrec[:, :qn].rearrange("(h g) f -> h g f", g=32)
                nc.sync.dma_start(out=zzv_[:, 0, :], in_=zrv_[:, 2, :])
                nc.gpsimd.dma_start(out=zzv_[:, 1, :], in_=zrv_[:, 2, :])
                osp = upool.tile([128, 512], F32, tag="osp", name="osp")
                nc.vector.tensor_mul(osp[:, :qn], usrc, zz[:, :qn])
                ospv = osp[:, :qn].rearrange("(h g) f -> h g f", g=32)
                otv = ot[0 : 2 * HLOC, qo : qo + qn].rearrange(
                    "(h g) f -> h g f", g=2
                )
                nc.sync.dma_start(out=otv[:, 0, :], in_=ospv[:, 0, :])
                nc.gpsimd.dma_start(out=otv[:, 1, :], in_=ospv[:, 1, :])

            def proj_chunks(ts_):
                for t in ts_:
                    to, tn = TCS[t]
                    py = ps_s.tile([128, 1536], F32, tag="s", name="py")
                    nc.tensor.matmul(
                        py[:tn, 0:C], ot[:, to : to + tn], wp_sb[:],
                        start=True, stop=True,
                    )
                    nc.vector.tensor_copy(ybv[:tn, t, :], py[:tn, 0:C])

            def y_dma(lo, hi):
                c0, c1 = lo // 128, hi // 128
                yv = y[lo:hi, :].rearrange("(t i) c -> i t c", i=128)
                nc.sync.dma_start(out=yv, in_=ybv[:128, c0:c1, :])

            def qtile_loop(qi, boundary_work, units=UNITS):
                qo, qn = QTS[qi]
                pu = ps_u.tile([128, 512], F32, tag="pu", name="pu")
                prev = None
                for ui, unit in enumerate(units):
                    st = ps_s.tile([128, 1536], F32, tag="s", name="st")
                    for j, (kc, h) in enumerate(unit):
                        ko, kn = KCS[kc]
                        nc.tensor.matmul(
                            st[:kn, 512 * j : 512 * j + qn],
                            kT[32 * h : 32 * h + 2, ko : ko + kn],
                            qT[32 * h : 32 * h + 2, qo : qo + qn],
                            start=True, stop=True,
                            tile_position=(32 * h, 0),
                        )
                    e = epool.tile([128, 1536], BF16, tag="e", name="e")
                    ns = len(unit)
                    src = st[:].rearrange("p (s q) -> p s q", q=512)[:, 0:ns, 0:qn]
                    dst = e[:].rearrange("p (s q) -> p s q", q=512)[:, 0:ns, 0:qn]
                    nc.scalar.activation(dst, src, EXP, scale=SCALE)
                    if prev is not None:
                        emit_U(pu, prev[0], prev[1], qn)
                    work = boundary_work.get(ui)
                    if work:
                        work()
                    prev = (e, unit)
                emit_U(pu, prev[0], prev[1], qn)
                return pu

            # ---- qtile 0: qkv q/k first columns upfront (head 0 first so
            # the first unit's scores are ready ASAP); V' + remaining k/q
            # chunks at unit boundaries (in pairs, keeping ring parity) ----
            for h in range(HLOC):
                qkv_tile(wq_sl, qT, 0, 512, [h])
                qkv_tile(wk_sl, kT, 0, 512, [h])

            bw0 = {
                0: lambda: (emit_vprime_group(0), emit_vprime_group(1)),
                1: lambda: (emit_vprime_group(2), emit_vprime_group(3)),
                2: lambda: (qkv_tile(wk_sl, kT, 512, 512, [0, 1]),
                            qkv_tile(wk_sl, kT, 512, 512, [2, 3])),
                4: lambda: (qkv_tile(wk_sl, kT, 1024, 512, [0, 1]),
                            qkv_tile(wk_sl, kT, 1024, 512, [2, 3])),
                6: lambda: (qkv_tile(wk_sl, kT, 1536, 192, [0, 1]),
                            qkv_tile(wk_sl, kT, 1536, 192, [2, 3])),
                12: lambda: (qkv_tile(wq_sl, qT, 512, 512, [0, 1]),
                             qkv_tile(wq_sl, qT, 512, 512, [2, 3])),
            }
            pu_prev = qtile_loop(0, bw0)

            def mk_bw(qi, pu_p):
                qo_p = QTS[qi - 1][0]
                qn_p = QTS[qi - 1][1]
                c0 = qo_p // 128
                bw = {
                    1: lambda: divide_and_store(pu_p, qo_p, qn_p),
                    3: lambda: proj_chunks([c0, c0 + 1]),
                    4: lambda: proj_chunks([c0 + 2, c0 + 3]),
                    6: lambda: y_dma(qo_p, qo_p + qn_p),
                }
                if qi < 3:
                    qo_n = QTS[qi + 1][0]
                    qn_n = QTS[qi + 1][1]
                    bw[12] = lambda: (
                        qkv_tile(wq_sl, qT, qo_n, qn_n, [0, 1]),
                        qkv_tile(wq_sl, qT, qo_n, qn_n, [2, 3]),
                    )
                return bw

            for qi in range(1, 4):
                pu_cur = qtile_loop(qi, mk_bw(qi, pu_prev))
                pu_prev = pu_cur

            # tail: last qtile is only 192 tokens
            divide_and_store(pu_prev, 1536, 192, last=True)
            proj_chunks([12, 13])
            yv = y[1536:1664, :].rearrange("(t i) c -> i t c", i=128)
            nc.sync.dma_start(out=yv, in_=ybv[:128, 12:13, :])
            nc.gpsimd.dma_start(out=y[1664:1728, :], in_=ybv[:64, 13, :])

    return nc


_NC = None


def _get_nc():
    global _NC
    if _NC is None:
        _NC = build_nc()
        _NC.finalize()
    return _NC


def make_in_maps(x, w_qkv, w_proj, b_proj):
    x2 = np.ascontiguousarray(x.reshape(C, N)).astype(ml_dtypes.bfloat16)
    in_maps = []
    for c in range(NCORES):
        sl = slice(8 * c, 8 * c + 8)
        wq = w_qkv[sl, :].T
        wk = w_qkv[64 + 8 * c : 64 + 8 * c + 8, :].T
        wv = w_qkv[128 + 8 * c : 128 + 8 * c + 8, :].T
        wall = np.ascontiguousarray(
            np.concatenate([wq, wk, wv], axis=1)
        ).astype(ml_dtypes.bfloat16)
        wpm = np.concatenate(
            [w_proj[:, sl].T, (b_proj / NCORES)[None, :]], axis=0
        ).astype(np.float32)
        in_maps.append(
            {"x2": x2, "wqkv": wall, "wp": np.ascontiguousarray(wpm)}
        )
    return in_maps


def run(x, w_qkv, w_proj, b_proj, trace=False, **kw):
    nc = _get_nc()
    in_maps = make_in_maps(x, w_qkv, w_proj, b_proj)
    res = run_bass_kernel_spmd(
        nc, in_maps, core_ids=list(range(NCORES)), trace=trace, **kw
    )
    y = np.zeros((N, C), np.float32)
    for r in res.results:
        y += r["y"]
    return y.reshape(1, 12, 12, 12, C), res


def kernel(x, w_qkv, w_proj, b_proj):
    out, _ = run(
        np.asarray(x), np.asarray(w_qkv), np.asarray(w_proj), np.asarray(b_proj)
    )
    return out


# revision 24
# speedup vs baseline: 1.8674x; 1.1617x over previous
"""Trainium2 Bass kernel for nn_Attention (B=1, C=64, 12x12x12 spatial, 32 heads, head_dim=2).

Sharding: 32 heads split across 8 cores (4 heads/core). Each core computes
qkv projection for its heads, head-local attention (flash-style: S^T chunks
-> exp on ScalarE -> U/Z accumulation via matmul with V'=[V,1]), divides,
then applies its slice of w_proj rows to produce a partial output summed on
the host (tensor-parallel unshard) with bias/8 folded per core.

The kernel is ScalarE(exp)-bound: 4 heads x 1728^2 scores = 11.9M exps per
core at 1 elem/cycle/lane (~78us of pure FD time).  Structure:

 - Work is a stream of (key-chunk, head) "slots", each a [kn<=128, qn=512]
   score tile (one PSUM bank).  Slots are grouped 3 per "unit" = one
   [128, 1536] PSUM tile; ONE ACTIVATE per unit (FD=1536) amortizes the
   ~260ns per-instruction overhead that dominated the per-(kc,h) version.
 - The 3 S matmuls of a unit hit different PE row-strips (tile_position
   32h) and write disjoint banks of one tile, so they run concurrently;
   same for the U matmuls (col-strips).  PE stays well under the ACT rate.
 - PSUM: 2 unit buffers (2x3 banks) + 2 U accumulators (2x1 bank) = 8.
   Every loop boundary allocates PSUM scratch tiles in PAIRS so
   consecutive units always land on different ring buffers.
 - Query dim is processed in tiles of 512 (x3) + 192; U/Z accumulate per
   qtile; divide + w_proj run at the next qtile's early boundaries.
 - Input DMAs go out on three queues in parallel; a tiny warmup exp
   triggers the ~2.7us ACT table load during the DMAs.

Uses bacc.Bacc (not plain Bass): its compile() runs
move_matmul_waits_to_ldweights + generate_event_semaphores, which the
TRN2 one-wait-per-instruction ISA constraint requires for Tile kernels.

Self-contained: hardcodes all shapes.
"""

import numpy as np
import ml_dtypes

import concourse.bass as bass
import concourse.bacc as bacc
import concourse.mybir as mybir
from concourse import tile
from concourse.bass_utils import run_bass_kernel_spmd

C = 64
N = 1728  # 12*12*12
NCORES = 8
HLOC = 4          # heads per core
SCALE = float(2.0 ** -0.5)

# key chunks: 13x128 + one padded 64+64 chunk (keys 1728:1792 are zero-pad:
# zero k columns -> score 0 -> E=1, and V' rows are zeroed -> contribute
# nothing to U or Z; keeps every S tile a full 128 rows so exp never reads
# uninitialized PSUM)
NK = 1792  # padded key count
KCS = [(i * 128, 128) for i in range(14)]
NKC = len(KCS)
QTS = [(0, 512), (512, 512), (1024, 512), (1536, 192)]
SLOTS = [(kc, h) for kc in range(NKC) for h in range(HLOC)]  # 56
UNITS = [SLOTS[i : i + 3] for i in range(0, len(SLOTS), 3)]  # 18x3 + 1x2
# proj token chunks of 128 (last 64)
TCS = [(i * 128, 128) for i in range(13)] + [(1664, 64)]

F32 = mybir.dt.float32
BF16 = mybir.dt.bfloat16
EXP = mybir.ActivationFunctionType.Exp


def build_nc():
    nc = bacc.Bacc(None)

    x2 = nc.declare_dram_parameter("x2", [C, N], BF16, isOutput=False)
    wqkv = nc.declare_dram_parameter("wqkv", [C, 6 * HLOC], BF16, isOutput=False)
    wp = nc.declare_dram_parameter("wp", [2 * HLOC + 1, C], F32, isOutput=False)
    y = nc.declare_dram_parameter("y", [N, C], F32, isOutput=True)

    with tile.TileContext(nc) as tc:
        with (
            tc.tile_pool(name="const", bufs=1) as cpool,
            tc.tile_pool(name="epool", bufs=4) as epool,
            tc.tile_pool(name="upool", bufs=2) as upool,
            tc.tile_pool(name="ps_s", bufs=2, space=bass.MemorySpace.PSUM) as ps_s,
            tc.tile_pool(name="ps_u", bufs=1, space=bass.MemorySpace.PSUM) as ps_u,
            tc.tile_pool(name="ps_w", bufs=1, space=bass.MemorySpace.PSUM) as ps_w,
        ):
            x_sb = cpool.tile([C, N], BF16, name="x_sb")
            w_sb = cpool.tile([C, 6 * HLOC], BF16, name="w_sb")
            wp_sb = cpool.tile([2 * HLOC + 1, C], F32, name="wp_sb")
            qT = cpool.tile([128, N], BF16, name="qT")
            kT = cpool.tile([128, NK], BF16, name="kT")
            vp = cpool.tile([128, NKC * 3 * HLOC], BF16, name="vp")
            ot = cpool.tile([2 * HLOC + 1, N], F32, name="ot")
            ybig = cpool.tile([128, len(TCS) * C], F32, name="ybig")
            ybv = ybig[:].rearrange("p (t c) -> p t c", c=C)
            wrm = cpool.tile([1, 8], BF16, name="wrm")

            # ACT table warmup: the ~2.7us exp table load runs during DMAs
            nc.gpsimd.memset(wrm[:], 0.0)
            nc.scalar.activation(wrm[:], wrm[:], EXP)

            # input DMAs on three queues in parallel; first x chunk gates
            # the first S matmuls, so it goes out first on its own queue
            nc.sync.dma_start(out=x_sb[:, 0:512], in_=x2[:, 0:512])
            nc.sync.dma_start(out=x_sb[:, 512:1024], in_=x2[:, 512:1024])
            nc.scalar.dma_start(out=x_sb[:, 1024:N], in_=x2[:, 1024:N])
            nc.gpsimd.dma_start(out=w_sb[:], in_=wqkv[:])
            nc.gpsimd.dma_start(out=wp_sb[:], in_=wp[:])

            # ones row for proj bias (rows 0..7 overwritten by attention out)
            nc.gpsimd.memset(ot[:, :], 1.0)
            vp_v = vp[:].rearrange("p (a b c) -> p a b c", b=HLOC, c=3)
            nc.gpsimd.memset(vp_v[:, :, :, 2:3], 1.0)
            # zero-pad: k columns for pad keys and V' pad rows of last chunk
            nc.gpsimd.memset(kT[:, N:NK], 0.0)
            nc.gpsimd.memset(vp_v[64:128, NKC - 1 : NKC, :, :], 0.0)

            wq_sl = w_sb[:, 0 : 2 * HLOC]
            wk_sl = w_sb[:, 2 * HLOC : 4 * HLOC]
            wv_sl = w_sb[:, 4 * HLOC : 6 * HLOC]

            # ---- V' in groups of 4 key chunks; ONE ps_w scratch slot each.
            # The last chunk only has 64 real keys (pad rows stay zero). ----
            def emit_vprime_group(g):
                kcs = list(range(4 * g, min(4 * g + 4, NKC)))
                psv = ps_w.tile([128, 512], F32, tag="w", name="ps_v")
                rows = 128
                for i, kc in enumerate(kcs):
                    ko, kn = KCS[kc]
                    kr = min(kn, N - ko)  # real (non-pad) keys
                    rows = min(rows, kr)
                    nc.tensor.matmul(
                        psv[:kr, 8 * i : 8 * i + 2 * HLOC],
                        x_sb[:, ko : ko + kr],
                        wv_sl,
                        start=True, stop=True,
                    )
                vsrc = psv[:rows, 0 : 8 * len(kcs)].rearrange(
                    "p (kc h d) -> p kc h d", h=HLOC, d=2
                )
                nc.vector.tensor_copy(
                    vp_v[:rows, 4 * g : 4 * g + len(kcs), :, 0:2], vsrc
                )
                if rows < 128:
                    # full-row chunks of this group copied separately
                    vsrc2 = psv[rows:128, 0 : 8 * (len(kcs) - 1)].rearrange(
                        "p (kc h d) -> p kc h d", h=HLOC, d=2
                    )
                    nc.vector.tensor_copy(
                        vp_v[rows:128, 4 * g : 4 * g + len(kcs) - 1, :, 0:2],
                        vsrc2,
                    )

            def qkv_tile(w_sl, dst, off, qn, heads, pool=None, cp="dve"):
                """Per-head matmuls (rows at partitions 32h) + copies.
                One scratch-ring slot per call (ps_w by default; the
                startup calls use the then-idle ps_s ring and ScalarE
                copies so the chain to the first exp is short)."""
                if pool is None:
                    ps = ps_w.tile([128, 512], F32, tag="w", name="ps_qkv")
                else:
                    ps = pool.tile([128, 1536], F32, tag="s", name="ps_qkv")
                for h in heads:
                    nc.tensor.matmul(
                        ps[32 * h : 32 * h + 2, 0:qn],
                        w_sl[:, 2 * h : 2 * h + 2],
                        x_sb[:, off : off + qn],
                        start=True, stop=True,
                        tile_position=(0, 32 * h),
                    )
                    if cp == "act":
                        nc.scalar.activation(
                            dst[32 * h : 32 * h + 2, off : off + qn],
                            ps[32 * h : 32 * h + 2, :qn],
                            mybir.ActivationFunctionType.Copy,
                        )
                    else:
                        nc.vector.tensor_copy(
                            dst[32 * h : 32 * h + 2, off : off + qn],
                            ps[32 * h : 32 * h + 2, :qn],
                        )

            def emit_U(pu, e, unit, qn):
                for j, (kc, h) in enumerate(unit):
                    nc.tensor.matmul(
                        pu[32 * h : 32 * h + 3, 0:qn],
                        vp_v[:, kc, h, :],
                        e[:, 512 * j : 512 * j + qn],
                        start=(kc == 0), stop=(kc == NKC - 1),
                        tile_position=(0, 32 * h),
                    )

            def divide_and_store(pu, qo, qn, last=False):
                """O^T rows 2h+d of `ot` <- U rows / Z row (per head)."""
                if last:
                    usrc = pu[:, :qn]
                else:
                    u_sb = upool.tile([128, 512], F32, tag="u_sb", name="u_sb")
                    nc.vector.tensor_copy(u_sb[:, :qn], pu[:, :qn])
                    usrc = u_sb[:, :qn]
                zrec = upool.tile([128, 512], F32, tag="zrec", name="zrec")
                nc.vector.reciprocal(zrec[:, :qn], usrc)
                zz = upool.tile([128, 512], F32, tag="zz", name="zz")
                zzv_ = zz[:, :qn].rearrange("(h g) f -> h g f", g=32)
                zrv_ = zrec[:, :qn].rearrange("(h g) f -> h g f", g=32)
                nc.sync.dma_start(out=zzv_[:, 0, :], in_=zrv_[:, 2, :])
                nc.gpsimd.dma_start(out=zzv_[:, 1, :], in_=zrv_[:, 2, :])
                osp = upool.tile([128, 512], F32, tag="osp", name="osp")
                nc.vector.tensor_mul(osp[:, :qn], usrc, zz[:, :qn])
                ospv = osp[:, :qn].rearrange("(h g) f -> h g f", g=32)
                otv = ot[0 : 2 * HLOC, qo : qo + qn].rearrange(
                    "(h g) f -> h g f", g=2
                )
                nc.sync.dma_start(out=otv[:, 0, :], in_=ospv[:, 0, :])
                nc.gpsimd.dma_start(out=otv[:, 1, :], in_=ospv[:, 1, :])

            def proj_chunks(ts_):
                for t in ts_:
                    to, tn = TCS[t]
                    py = ps_s.tile([128, 1536], F32, tag="s", name="py")
                    nc.tensor.matmul(
                        py[:tn, 0:C], ot[:, to : to + tn], wp_sb[:],
                        start=True, stop=True,
                    )
                    nc.vector.tensor_copy(ybv[:tn, t, :], py[:tn, 0:C])

            def y_dma(lo, hi):
                c0, c1 = lo // 128, hi // 128
                yv = y[lo:hi, :].rearrange("(t i) c -> i t c", i=128)
                nc.sync.dma_start(out=yv, in_=ybv[:128, c0:c1, :])

            pus = []

            def attention_stream(boundary_work):
                """One continuous stream of (qtile, unit) work: U always one
                unit behind S/exp, ALSO across qtile transitions, so the
                next qtile's scores are in flight before the previous
                qtile's last exp finishes.  Appends the per-qtile pu tiles
                to `pus` (all but the last are consumed by boundary
                divides)."""
                prev = None  # (pu, e, unit, qn)
                for qi, (qo, qn) in enumerate(QTS):
                    pu = ps_u.tile([128, 512], F32, tag="pu", name="pu")
                    pus.append(pu)
                    for ui, unit in enumerate(UNITS):
                        st = ps_s.tile([128, 1536], F32, tag="s", name="st")
                        for j, (kc, h) in enumerate(unit):
                            ko, kn = KCS[kc]
                            nc.tensor.matmul(
                                st[:, 512 * j : 512 * j + qn],
                                kT[32 * h : 32 * h + 2, ko : ko + kn],
                                qT[32 * h : 32 * h + 2, qo : qo + qn],
                                start=True, stop=True,
                                tile_position=(32 * h, 0),
                            )
                        e = epool.tile([128, 1536], BF16, tag="e", name="e")
                        ns = len(unit)
                        src = st[:].rearrange("p (s q) -> p s q", q=512)[
                            :, 0:ns, 0:qn
                        ]
                        dst = e[:].rearrange("p (s q) -> p s q", q=512)[
                            :, 0:ns, 0:qn
                        ]
                        nc.scalar.activation(dst, src, EXP, scale=SCALE)
                        if prev is not None:
                            emit_U(*prev)
                        work = boundary_work.get((qi, ui))
                        if work:
                            work()
                        prev = (pu, e, unit, qn)
                emit_U(*prev)

            # ---- startup qkv: q/k first 512 columns for all heads, on the
            # then-idle ps_s ring; q copies on ScalarE, k copies on DVE so
            # the serial copy chain to the first exp is halved ----
            for h in range(HLOC):
                qkv_tile(wq_sl, qT, 0, 512, [h], pool=ps_s, cp="act")
                qkv_tile(wk_sl, kT, 0, 512, [h], pool=ps_s, cp="dve")

            # boundary work: one psum-scratch allocation per boundary max
            bw = {
                (0, 0): lambda: emit_vprime_group(0),
                (0, 1): lambda: emit_vprime_group(1),
                (0, 2): lambda: qkv_tile(wk_sl, kT, 512, 512, [0, 1]),
                (0, 3): lambda: qkv_tile(wk_sl, kT, 512, 512, [2, 3]),
                (0, 4): lambda: emit_vprime_group(2),
                (0, 5): lambda: emit_vprime_group(3),
                (0, 6): lambda: qkv_tile(wk_sl, kT, 1024, 512, [0, 1]),
                (0, 7): lambda: qkv_tile(wk_sl, kT, 1024, 512, [2, 3]),
                (0, 9): lambda: qkv_tile(wk_sl, kT, 1536, 192, [0, 1]),
                (0, 10): lambda: qkv_tile(wk_sl, kT, 1536, 192, [2, 3]),
                (0, 13): lambda: qkv_tile(wq_sl, qT, 512, 512, [0, 1]),
                (0, 14): lambda: qkv_tile(wq_sl, qT, 512, 512, [2, 3]),
                (1, 13): lambda: qkv_tile(wq_sl, qT, 1024, 512, [0, 1]),
                (1, 14): lambda: qkv_tile(wq_sl, qT, 1024, 512, [2, 3]),
                (2, 13): lambda: qkv_tile(wq_sl, qT, 1536, 192, [0, 1]),
                (2, 14): lambda: qkv_tile(wq_sl, qT, 1536, 192, [2, 3]),
            }
            def add_qt_bw(qi):
                # divide at unit 0 (right after the previous qtile's U
                # stops); proj waits until its ot values have landed, so
                # the proj matmuls never stall the PE FIFO
                qo_p, qn_p = QTS[qi - 1]
                c0 = qo_p // 128
                bw[(qi, 0)] = lambda: divide_and_store(pus[qi - 1], qo_p, qn_p)
                for k in range(qn_p // 128):
                    bw[(qi, 8 + k)] = lambda t=c0 + k: proj_chunks([t])
                bw[(qi, 12)] = lambda: y_dma(qo_p, qo_p + qn_p)

            for qi in range(1, 4):
                add_qt_bw(qi)

            attention_stream(bw)

            # tail: last qtile is only 192 tokens
            divide_and_store(pus[3], 1536, 192, last=True)
            proj_chunks([12, 13])
            yv = y[1536:1664, :].rearrange("(t i) c -> i t c", i=128)
            nc.sync.dma_start(out=yv, in_=ybv[:128, 12:13, :])
            nc.gpsimd.dma_start(out=y[1664:1728, :], in_=ybv[:64, 13, :])

    return nc


_NC = None


def _get_nc():
    global _NC
    if _NC is None:
        _NC = build_nc()
        _NC.finalize()
    return _NC


def make_in_maps(x, w_qkv, w_proj, b_proj):
    x2 = np.ascontiguousarray(x.reshape(C, N)).astype(ml_dtypes.bfloat16)
    in_maps = []
    for c in range(NCORES):
        sl = slice(8 * c, 8 * c + 8)
        wq = w_qkv[sl, :].T
        wk = w_qkv[64 + 8 * c : 64 + 8 * c + 8, :].T
        wv = w_qkv[128 + 8 * c : 128 + 8 * c + 8, :].T
        wall = np.ascontiguousarray(
            np.concatenate([wq, wk, wv], axis=1)
        ).astype(ml_dtypes.bfloat16)
        wpm = np.concatenate(
            [w_proj[:, sl].T, (b_proj / NCORES)[None, :]], axis=0
        ).astype(np.float32)
        in_maps.append(
            {"x2": x2, "wqkv": wall, "wp": np.ascontiguousarray(wpm)}
        )
    return in_maps


def run(x, w_qkv, w_proj, b_proj, trace=False, **kw):
    nc = _get_nc()
    in_maps = make_in_maps(x, w_qkv, w_proj, b_proj)
    res = run_bass_kernel_spmd(
        nc, in_maps, core_ids=list(range(NCORES)), trace=trace, **kw
    )
    y = np.zeros((N, C), np.float32)
    for r in res.results:
        y += r["y"]
    return y.reshape(1, 12, 12, 12, C), res


def kernel(x, w_qkv, w_proj, b_proj):
    out, _ = run(
        np.asarray(x), np.asarray(w_qkv), np.asarray(w_proj), np.asarray(b_proj)
    )
    return out


# revision 31
# speedup vs baseline: 1.9280x; 1.0325x over previous
"""Trainium2 Bass kernel for nn_Attention (B=1, C=64, 12x12x12 spatial, 32 heads, head_dim=2).

Sharding: 32 heads split across 8 cores (4 heads/core). Each core computes
qkv projection for its heads, head-local attention (flash-style: S^T chunks
-> exp on ScalarE -> U/Z accumulation via matmul with V'=[V,1]), divides,
then applies its slice of w_proj rows to produce a partial output summed on
the host (tensor-parallel unshard) with bias/8 folded per core.

The kernel is ScalarE(exp)-bound: 4 heads x 1728^2 scores = 11.9M exps per
core at 1 elem/cycle/lane (~78us of pure FD time).  Structure:

 - Work is a stream of (key-chunk, head) "slots", each a [kn<=128, qn=512]
   score tile (one PSUM bank).  Slots are grouped 3 per "unit" = one
   [128, 1536] PSUM tile; ONE ACTIVATE per unit (FD=1536) amortizes the
   ~260ns per-instruction overhead that dominated the per-(kc,h) version.
 - The 3 S matmuls of a unit hit different PE row-strips (tile_position
   32h) and write disjoint banks of one tile, so they run concurrently;
   same for the U matmuls (col-strips).  PE stays well under the ACT rate.
 - PSUM: 2 unit buffers (2x3 banks) + 2 U accumulators (2x1 bank) = 8.
   Every loop boundary allocates PSUM scratch tiles in PAIRS so
   consecutive units always land on different ring buffers.
 - Query dim is processed in tiles of 512 (x3) + 192; U/Z accumulate per
   qtile; divide + w_proj run at the next qtile's early boundaries.
 - Input DMAs go out on three queues in parallel; a tiny warmup exp
   triggers the ~2.7us ACT table load during the DMAs.

Uses bacc.Bacc (not plain Bass): its compile() runs
move_matmul_waits_to_ldweights + generate_event_semaphores, which the
TRN2 one-wait-per-instruction ISA constraint requires for Tile kernels.

Self-contained: hardcodes all shapes.
"""

import numpy as np
import ml_dtypes

import concourse.bass as bass
import concourse.bacc as bacc
import concourse.mybir as mybir
from concourse import tile
from concourse.bass_utils import run_bass_kernel_spmd

C = 64
N = 1728  # 12*12*12
NCORES = 8
HLOC = 4          # heads per core
SCALE = float(2.0 ** -0.5)

# key chunks: 13x128 + one padded 64+64 chunk (keys 1728:1792 are zero-pad:
# zero k columns -> score 0 -> E=1, and V' rows are zeroed -> contribute
# nothing to U or Z; keeps every S tile a full 128 rows so exp never reads
# uninitialized PSUM)
NK = 1792  # padded key count
KCS = [(i * 128, 128) for i in range(14)]
NKC = len(KCS)
QTS = [(0, 512), (512, 512), (1024, 512), (1536, 192)]
SLOTS = [(kc, h) for kc in range(NKC) for h in range(HLOC)]  # 56
UNITS = [SLOTS[i : i + 3] for i in range(0, len(SLOTS), 3)]  # 18x3 + 1x2
# NOTE: packing >1 slot per PSUM bank (e.g. 256-col pitch) is NOT safe:
# the concurrent S matmuls of a unit would write the same bank, which is
# a fatal PSUM hazard on TRN2.  All qtiles use one bank per slot.
QT_UNITS = [(UNITS, 512), (UNITS, 512), (UNITS, 512), (UNITS, 512)]
# proj token chunks of 128 (last 64)
TCS = [(i * 128, 128) for i in range(13)] + [(1664, 64)]

F32 = mybir.dt.float32
BF16 = mybir.dt.bfloat16
EXP = mybir.ActivationFunctionType.Exp


def build_nc():
    nc = bacc.Bacc(None)

    x2 = nc.declare_dram_parameter("x2", [C, N], BF16, isOutput=False)
    wqkv = nc.declare_dram_parameter("wqkv", [C, 6 * HLOC], BF16, isOutput=False)
    wp = nc.declare_dram_parameter("wp", [2 * HLOC + 1, C], F32, isOutput=False)
    y = nc.declare_dram_parameter("y", [N, C], F32, isOutput=True)

    with tile.TileContext(nc) as tc:
        with (
            tc.tile_pool(name="const", bufs=1) as cpool,
            tc.tile_pool(name="epool", bufs=4) as epool,
            tc.tile_pool(name="upool", bufs=2) as upool,
            tc.tile_pool(name="ps_s", bufs=2, space=bass.MemorySpace.PSUM) as ps_s,
            tc.tile_pool(name="ps_u", bufs=1, space=bass.MemorySpace.PSUM) as ps_u,
            tc.tile_pool(name="ps_w", bufs=1, space=bass.MemorySpace.PSUM) as ps_w,
        ):
            x_sb = cpool.tile([C, N], BF16, name="x_sb")
            w_sb = cpool.tile([C, 6 * HLOC], BF16, name="w_sb")
            wp_sb = cpool.tile([2 * HLOC + 1, C], F32, name="wp_sb")
            qT = cpool.tile([128, N], BF16, name="qT")
            kT = cpool.tile([128, NK], BF16, name="kT")
            vp = cpool.tile([128, NKC * 3 * HLOC], BF16, name="vp")
            ot = cpool.tile([2 * HLOC + 1, N], F32, name="ot")
            ybig = cpool.tile([128, len(TCS) * C], F32, name="ybig")
            ybv = ybig[:].rearrange("p (t c) -> p t c", c=C)
            wrm = cpool.tile([1, 8], BF16, name="wrm")

            # ACT table warmup: the ~2.7us exp table load runs during DMAs
            nc.gpsimd.memset(wrm[:], 0.0)
            nc.scalar.activation(wrm[:], wrm[:], EXP)

            # input DMAs on three queues in parallel; first x chunk gates
            # the first S matmuls, so it goes out first on its own queue
            nc.sync.dma_start(out=x_sb[:, 0:512], in_=x2[:, 0:512])
            nc.sync.dma_start(out=x_sb[:, 512:1024], in_=x2[:, 512:1024])
            nc.scalar.dma_start(out=x_sb[:, 1024:N], in_=x2[:, 1024:N])
            nc.gpsimd.dma_start(out=w_sb[:], in_=wqkv[:])
            nc.gpsimd.dma_start(out=wp_sb[:], in_=wp[:])

            # ones row for proj bias (rows 0..7 overwritten by attention out)
            nc.gpsimd.memset(ot[:, :], 1.0)
            vp_v = vp[:].rearrange("p (a b c) -> p a b c", b=HLOC, c=3)
            nc.gpsimd.memset(vp_v[:, :, :, 2:3], 1.0)
            # zero-pad: k columns for pad keys and V' pad rows of last chunk
            nc.gpsimd.memset(kT[:, N:NK], 0.0)
            nc.gpsimd.memset(vp_v[64:128, NKC - 1 : NKC, :, :], 0.0)

            wq_sl = w_sb[:, 0 : 2 * HLOC]
            wk_sl = w_sb[:, 2 * HLOC : 4 * HLOC]
            wv_sl = w_sb[:, 4 * HLOC : 6 * HLOC]

            # ---- V' in groups of 4 key chunks; ONE ps_w scratch slot each.
            # The last chunk only has 64 real keys (pad rows stay zero). ----
            def emit_vprime_group(g):
                kcs = list(range(4 * g, min(4 * g + 4, NKC)))
                psv = ps_w.tile([128, 512], F32, tag="w", name="ps_v")
                rows = 128
                for i, kc in enumerate(kcs):
                    ko, kn = KCS[kc]
                    kr = min(kn, N - ko)  # real (non-pad) keys
                    rows = min(rows, kr)
                    nc.tensor.matmul(
                        psv[:kr, 8 * i : 8 * i + 2 * HLOC],
                        x_sb[:, ko : ko + kr],
                        wv_sl,
                        start=True, stop=True,
                    )
                vsrc = psv[:rows, 0 : 8 * len(kcs)].rearrange(
                    "p (kc h d) -> p kc h d", h=HLOC, d=2
                )
                nc.vector.tensor_copy(
                    vp_v[:rows, 4 * g : 4 * g + len(kcs), :, 0:2], vsrc
                )
                if rows < 128:
                    # full-row chunks of this group copied separately
                    vsrc2 = psv[rows:128, 0 : 8 * (len(kcs) - 1)].rearrange(
                        "p (kc h d) -> p kc h d", h=HLOC, d=2
                    )
                    nc.vector.tensor_copy(
                        vp_v[rows:128, 4 * g : 4 * g + len(kcs) - 1, :, 0:2],
                        vsrc2,
                    )

            def qkv_tile(w_sl, dst, off, qn, heads, pool=None, cp="dve"):
                """Per-head matmuls (rows at partitions 32h) + copies.
                One scratch-ring slot per call (ps_w by default; the
                startup calls use the then-idle ps_s ring and ScalarE
                copies so the chain to the first exp is short)."""
                if pool is None:
                    ps = ps_w.tile([128, 512], F32, tag="w", name="ps_qkv")
                else:
                    ps = pool.tile([128, 1536], F32, tag="s", name="ps_qkv")
                for h in heads:
                    nc.tensor.matmul(
                        ps[32 * h : 32 * h + 2, 0:qn],
                        w_sl[:, 2 * h : 2 * h + 2],
                        x_sb[:, off : off + qn],
                        start=True, stop=True,
                        tile_position=(0, 32 * h),
                    )
                    if cp == "act":
                        nc.scalar.activation(
                            dst[32 * h : 32 * h + 2, off : off + qn],
                            ps[32 * h : 32 * h + 2, :qn],
                            mybir.ActivationFunctionType.Copy,
                        )
                    else:
                        nc.vector.tensor_copy(
                            dst[32 * h : 32 * h + 2, off : off + qn],
                            ps[32 * h : 32 * h + 2, :qn],
                        )

            def emit_U(pu, e, unit, qn, pitch):
                for j, (kc, h) in enumerate(unit):
                    nc.tensor.matmul(
                        pu[32 * h : 32 * h + 3, 0:qn],
                        vp_v[:, kc, h, :],
                        e[:, pitch * j : pitch * j + qn],
                        start=(kc == 0), stop=(kc == NKC - 1),
                        tile_position=(0, 32 * h),
                    )

            def divide_and_store(pu, qo, qn, last=False):
                """O^T rows 2h+d of `ot` <- U rows / Z row (per head)."""
                if last:
                    usrc = pu[:, :qn]
                else:
                    u_sb = upool.tile([128, 512], F32, tag="u_sb", name="u_sb")
                    nc.vector.tensor_copy(u_sb[:, :qn], pu[:, :qn])
                    usrc = u_sb[:, :qn]
                zrec = upool.tile([128, 512], F32, tag="zrec", name="zrec")
                nc.vector.reciprocal(zrec[:, :qn], usrc)
                zz = upool.tile([128, 512], F32, tag="zz", name="zz")
                zzv_ = zz[:, :qn].rearrange("(h g) f -> h g f", g=32)
                zrv_ = zrec[:, :qn].rearrange("(h g) f -> h g f", g=32)
                nc.sync.dma_start(out=zzv_[:, 0, :], in_=zrv_[:, 2, :])
                nc.gpsimd.dma_start(out=zzv_[:, 1, :], in_=zrv_[:, 2, :])
                osp = upool.tile([128, 512], F32, tag="osp", name="osp")
                nc.vector.tensor_mul(osp[:, :qn], usrc, zz[:, :qn])
                ospv = osp[:, :qn].rearrange("(h g) f -> h g f", g=32)
                otv = ot[0 : 2 * HLOC, qo : qo + qn].rearrange(
                    "(h g) f -> h g f", g=2
                )
                nc.sync.dma_start(out=otv[:, 0, :], in_=ospv[:, 0, :])
                nc.gpsimd.dma_start(out=otv[:, 1, :], in_=ospv[:, 1, :])

            def proj_chunks(ts_):
                for t in ts_:
                    to, tn = TCS[t]
                    py = ps_s.tile([128, 1536], F32, tag="s", name="py")
                    nc.tensor.matmul(
                        py[:tn, 0:C], ot[:, to : to + tn], wp_sb[:],
                        start=True, stop=True,
                    )
                    nc.vector.tensor_copy(ybv[:tn, t, :], py[:tn, 0:C])

            def y_dma(lo, hi):
                c0, c1 = lo // 128, hi // 128
                yv = y[lo:hi, :].rearrange("(t i) c -> i t c", i=128)
                nc.sync.dma_start(out=yv, in_=ybv[:128, c0:c1, :])

            pus = []

            def attention_stream(boundary_work):
                """One continuous stream of (qtile, unit) work: U always one
                unit behind S/exp, ALSO across qtile transitions, so the
                next qtile's scores are in flight before the previous
                qtile's last exp finishes.  Appends the per-qtile pu tiles
                to `pus` (all but the last are consumed by boundary
                divides)."""
                prev = None  # (pu, e, unit, qn, pitch)
                for qi, (qo, qn) in enumerate(QTS):
                    units, pitch = QT_UNITS[qi]
                    pu = ps_u.tile([128, 512], F32, tag="pu", name="pu")
                    pus.append(pu)
                    for ui, unit in enumerate(units):
                        st = ps_s.tile([128, 1536], F32, tag="s", name="st")
                        for j, (kc, h) in enumerate(unit):
                            ko, kn = KCS[kc]
                            nc.tensor.matmul(
                                st[:, pitch * j : pitch * j + qn],
                                kT[32 * h : 32 * h + 2, ko : ko + kn],
                                qT[32 * h : 32 * h + 2, qo : qo + qn],
                                start=True, stop=True,
                                tile_position=(32 * h, 0),
                            )
                        e = epool.tile([128, 1536], BF16, tag="e", name="e")
                        ns = len(unit)
                        src = st[:].rearrange("p (s q) -> p s q", q=pitch)[
                            :, 0:ns, 0:qn
                        ]
                        dst = e[:].rearrange("p (s q) -> p s q", q=pitch)[
                            :, 0:ns, 0:qn
                        ]
                        nc.scalar.activation(dst, src, EXP, scale=SCALE)
                        if prev is not None:
                            emit_U(*prev)
                        work = boundary_work.get((qi, ui))
                        if work:
                            work()
                        prev = (pu, e, unit, qn, pitch)
                emit_U(*prev)

            # ---- startup qkv: q/k first 512 columns for all heads, on the
            # then-idle ps_s ring; q copies on ScalarE, k copies on DVE so
            # the serial copy chain to the first exp is halved ----
            for h in range(HLOC):
                qkv_tile(wq_sl, qT, 0, 512, [h], pool=ps_s, cp="act")
                qkv_tile(wk_sl, kT, 0, 512, [h], pool=ps_s, cp="dve")

            # boundary work: one psum-scratch allocation per boundary max
            bw = {
                (0, 0): lambda: emit_vprime_group(0),
                (0, 1): lambda: emit_vprime_group(1),
                (0, 2): lambda: qkv_tile(wk_sl, kT, 512, 512, [0, 1]),
                (0, 3): lambda: qkv_tile(wk_sl, kT, 512, 512, [2, 3]),
                (0, 4): lambda: emit_vprime_group(2),
                (0, 5): lambda: emit_vprime_group(3),
                (0, 6): lambda: qkv_tile(wk_sl, kT, 1024, 512, [0, 1]),
                (0, 7): lambda: qkv_tile(wk_sl, kT, 1024, 512, [2, 3]),
                (0, 9): lambda: qkv_tile(wk_sl, kT, 1536, 192, [0, 1]),
                (0, 10): lambda: qkv_tile(wk_sl, kT, 1536, 192, [2, 3]),
                (0, 13): lambda: qkv_tile(wq_sl, qT, 512, 512, [0, 1]),
                (0, 14): lambda: qkv_tile(wq_sl, qT, 512, 512, [2, 3]),
                (1, 13): lambda: qkv_tile(wq_sl, qT, 1024, 512, [0, 1]),
                (1, 14): lambda: qkv_tile(wq_sl, qT, 1024, 512, [2, 3]),
                (2, 13): lambda: qkv_tile(wq_sl, qT, 1536, 192, [0, 1]),
                (2, 14): lambda: qkv_tile(wq_sl, qT, 1536, 192, [2, 3]),
            }
            def add_qt_bw(qi):
                # divide at unit 0 (right after the previous qtile's U
                # stops); proj waits until its ot values have landed, so
                # the proj matmuls never stall the PE FIFO.  qtile 3 only
                # has 10 (packed) units, so its schedule is compressed and
                # the last y DMA moves to the tail.
                qo_p, qn_p = QTS[qi - 1]
                c0 = qo_p // 128
                p0 = 8 if qi < 3 else 10
                bw[(qi, 0)] = lambda: divide_and_store(pus[qi - 1], qo_p, qn_p)
                for k in range(qn_p // 128):
                    bw[(qi, p0 + k)] = lambda t=c0 + k: proj_chunks([t])
                if qi < 3:
                    bw[(qi, 12)] = lambda: y_dma(qo_p, qo_p + qn_p)

            for qi in range(1, 4):
                add_qt_bw(qi)

            attention_stream(bw)

            # tail: qtile2's y DMA, then divide + proj of the last 192 tokens
            y_dma(1024, 1536)
            divide_and_store(pus[3], 1536, 192, last=True)
            proj_chunks([12, 13])
            yv = y[1536:1664, :].rearrange("(t i) c -> i t c", i=128)
            nc.gpsimd.dma_start(out=yv, in_=ybv[:128, 12:13, :])
            nc.sync.dma_start(out=y[1664:1728, :], in_=ybv[:64, 13, :])

    return nc


_NC = None


def _get_nc():
    global _NC
    if _NC is None:
        _NC = build_nc()
        _NC.finalize()
    return _NC


def make_in_maps(x, w_qkv, w_proj, b_proj):
    x2 = np.ascontiguousarray(x.reshape(C, N)).astype(ml_dtypes.bfloat16)
    in_maps = []
    for c in range(NCORES):
        sl = slice(8 * c, 8 * c + 8)
        wq = w_qkv[sl, :].T
        wk = w_qkv[64 + 8 * c : 64 + 8 * c + 8, :].T
        wv = w_qkv[128 + 8 * c : 128 + 8 * c + 8, :].T
        wall = np.ascontiguousarray(
            np.concatenate([wq, wk, wv], axis=1)
        ).astype(ml_dtypes.bfloat16)
        wpm = np.concatenate(
            [w_proj[:, sl].T, (b_proj / NCORES)[None, :]], axis=0
        ).astype(np.float32)
        in_maps.append(
            {"x2": x2, "wqkv": wall, "wp": np.ascontiguousarray(wpm)}
        )
    return in_maps


def run(x, w_qkv, w_proj, b_proj, trace=False, **kw):
    nc = _get_nc()
    in_maps = make_in_maps(x, w_qkv, w_proj, b_proj)
    res = run_bass_kernel_spmd(
        nc, in_maps, core_ids=list(range(NCORES)), trace=trace, **kw
    )
    y = np.zeros((N, C), np.float32)
    for r in res.results:
        y += r["y"]
    return y.reshape(1, 12, 12, 12, C), res


def kernel(x, w_qkv, w_proj, b_proj):
    out, _ = run(
        np.asarray(x), np.asarray(w_qkv), np.asarray(w_proj), np.asarray(b_proj)
    )
    return out
